# revision 24
# baseline (speedup 1.0000x reference)
"""Trainium2 Bass kernel for causal self-attention with RoPE.

Problem: B=2, S=2048, H=1024, NH=16 heads (HD=64), fp32, causal mask,
rotary embeddings, out = softmax(rope(XWq)(rope(XWk))^T/8 + mask) (XWv) Wo^T.

Sharding (8 cores): tensor-parallel over heads x data-parallel over batch.
core i -> (batch b = i//4, head-group g = i%4 of 4 heads = 256 channels).
Each core computes its group's Q/K/V projections, RoPE, causal attention and
a partial output projection (attnout_g @ Wo_g^T); the host sums the 4 group
partials per batch.

Device-side layout (v2):
 - hidden states fed TRANSPOSED (XT [H, S]); projections contract over the
   partition dim. X/W/Wo/attention-output stay fp32r (full PE rate at the
   512/256-wide moving operands used here).
 - Q^T/K^T [c, s] in bf16 after RoPE; V tiles [k, head, HD+1] bf16 with a
   ones column appended so attn@V also yields the softmax denominator.
 - rotate_half is a constant signed-permutation matmul (bf16); RoPE is
   elementwise in [c, s] split across DVE (psum operand) and Pool (sbuf).
 - scores are computed TRANSPOSED (P^T [ks, qs]) per head into a kt-PAIR
   psum tile [128, 2, 512]; ONE exp (scale=0.125, no max-subtraction:
   scores are bounded ~ +-5 for this problem) covers both kt blocks,
   halving Activation-engine instruction count. exp output is bf16.
 - causal masking multiplies only the 128-wide diagonal triangle of each
   diagonal kt block (everything right of it is already causal); the
   triangle mask is one [128,128] table shared by all blocks.
 - softmax normalization: reciprocal of the denominator row (DVE), then a
   gpsimd partition_broadcast replicates it across 64 partitions, then one
   DVE tensor-tensor multiply writes the normalized attention output.
 - attention output [hd, qs] is exactly the lhsT the output projection
   wants; out = at^T wo accumulated over the 2 half-channel groups.
"""
import sys
import numpy as np

sys.path.insert(0, '/opt/trn_rl_repo')

B, S, H, NH, HD = 2, 2048, 1024, 16, 64
GROUPS = 4            # head-groups (cores per batch)
HEADS_PER_CORE = 4
C = HEADS_PER_CORE * HD   # 256 channels per core
P = 128               # partitions
SC = 512              # s-chunk (matmul free dim)
N_SCHUNK = S // SC    # 4
N_HT = H // P         # 8 h-tiles
N_ST = S // P         # 16 s-tiles
ROPE_THETA = 10000.0

_PROGRAM_CACHE = {}
TRACE = False          # set True (e.g. from test.py) to profile and fill LAST_EXEC_NS
LAST_EXEC_NS = None

USE_PBROADCAST = True  # gpsimd partition_broadcast for softmax normalization
MASK_ENGINE = "vector"    # engine for the diagonal triangle mask multiply
ROPE_COS_ENGINE = "gpsimd"  # engine for rope's qt*cos and +=rt ops


def _build_program(loop_n=None):
    import concourse.bass as bass
    import concourse.mybir as mybir
    import concourse.tile as tile
    from concourse import bacc

    f32 = mybir.dt.float32
    f32r = mybir.dt.float32r
    bf16 = mybir.dt.bfloat16
    Exp = mybir.ActivationFunctionType.Exp
    mult = mybir.AluOpType.mult
    add = mybir.AluOpType.add

    nc = bacc.Bacc("TRN2", target_bir_lowering=False, debug=False, num_devices=8)

    # ---- DRAM parameters (per-core inputs) ----
    hsT = nc.declare_dram_parameter("hsT", [H, S], f32, isOutput=False)
    wqT = nc.declare_dram_parameter("wqT", [H, C], f32, isOutput=False)
    wkT = nc.declare_dram_parameter("wkT", [H, C], f32, isOutput=False)
    wvT = nc.declare_dram_parameter("wvT", [H, C], f32, isOutput=False)
    woT = nc.declare_dram_parameter("woT", [C, H], f32, isOutput=False)
    cosT = nc.declare_dram_parameter("cosT", [P, S], bf16, isOutput=False)
    sinT = nc.declare_dram_parameter("sinT", [P, S], bf16, isOutput=False)
    tri = nc.declare_dram_parameter("tri", [P, P], bf16, isOutput=False)
    r2t = nc.declare_dram_parameter("r2t", [P, P], bf16, isOutput=False)
    onesv = nc.declare_dram_parameter("onesv", [1, P], f32, isOutput=False)
    vones = nc.declare_dram_parameter("vones", [P, N_ST * HEADS_PER_CORE], bf16,
                                      isOutput=False)
    out_d = nc.declare_dram_parameter("out", [S, H], f32, isOutput=True)

    mm = nc.tensor.matmul

    with tile.TileContext(nc) as tc, \
         nc.allow_low_precision(reason="bf16 attention internals; "
                                "accumulation stays fp32 in PSUM"):
        import contextlib
        stack = contextlib.ExitStack()
        persist = stack.enter_context(tc.tile_pool(name="persist", bufs=1))
        work = stack.enter_context(tc.tile_pool(name="work", bufs=1))
        psp = stack.enter_context(tc.tile_pool(name="psp", bufs=1, space="PSUM"))

        # ---------------- persistent tiles ----------------
        cos_sb = persist.tile([P, S], bf16, tag="cos")
        sin_sb = persist.tile([P, S], bf16, tag="sin")
        tri_sb = persist.tile([P, P], bf16, tag="tri")
        r2t_sb = persist.tile([P, P], bf16, tag="r2t")
        ones_sb = persist.tile([1, P], f32r, tag="ones")
        wo_sb = [persist.tile([P, H], f32r, tag=f"wo{i}", name=f"wo{i}") for i in range(2)]
        qr_sb = [persist.tile([P, S], bf16, tag=f"qr{i}", name=f"qr{i}") for i in range(2)]
        kr_sb = [persist.tile([P, S], bf16, tag=f"kr{i}", name=f"kr{i}") for i in range(2)]
        vx_sb = persist.tile([P, N_ST, HEADS_PER_CORE, HD + 1], bf16, tag="vx")
        at_sb = [persist.tile([P, S], f32r, tag=f"at{i}", name=f"at{i}") for i in range(2)]
        w_sb = {n: work.tile([P, N_HT, C], f32r, tag=f"w{n}", name=f"w{n}")
                for n in ("q", "k", "v")}

        xt_tiles = {}

        def load_xt(j, halves=(0, 1)):
            # one DMA per half chunk: [128, 4 h-tiles, 512] (rotating, bufs=2)
            if j in xt_tiles:
                x = xt_tiles[j]
            else:
                x = work.tile([P, N_HT, SC], f32r, tag="xt", name=f"xt_{j}",
                              bufs=2)
                xt_tiles[j] = x
            src = hsT.ap().rearrange("(t p) s -> p t s", p=P)
            for hf in halves:
                tsl = slice(hf * 4, hf * 4 + 4)
                nc.sync.dma_start(
                    out=x[:, tsl, :],
                    in_=src[:, tsl, j * SC:(j + 1) * SC].bitcast(f32r))

        # ---- startup DMAs: q-weights + first x chunk first, then the rest ----
        wdram = {"q": wqT, "k": wkT, "v": wvT}
        def emit_startup_dmas():
            def one_w(n, halves=(0, 1)):
                src = wdram[n].ap().rearrange("(t p) c -> p t c", p=P)
                for hf in halves:
                    tsl = slice(hf * 4, hf * 4 + 4)
                    nc.sync.dma_start(out=w_sb[n][:, tsl, :],
                                      in_=src[:, tsl, :].bitcast(f32r))
            # q projections need wq + the x chunk; rope needs the tables;
            # then k; xt(1) before wv so prep(1) can fill the startup hole.
            one_w("q", halves=(0,))
            load_xt(0, halves=(0,))
            one_w("q", halves=(1,))
            load_xt(0, halves=(1,))
            nc.sync.dma_start(out=r2t_sb[:], in_=r2t[:])
            nc.sync.dma_start(out=cos_sb[:], in_=cosT[:])
            nc.sync.dma_start(out=sin_sb[:], in_=sinT[:])
            one_w("k")
            nc.sync.dma_start(out=tri_sb[:], in_=tri[:])
            load_xt(1)
            nc.sync.dma_start(
                out=vx_sb[:, :, :, HD],
                in_=vones.ap().rearrange("p (st h) -> p st h", h=HEADS_PER_CORE))
            one_w("v")
            nc.sync.dma_start(out=ones_sb[:], in_=onesv.ap().bitcast(f32r))
            for i in range(2):
                nc.sync.dma_start(out=wo_sb[i][:],
                                  in_=woT[i * P:(i + 1) * P, :].bitcast(f32r))

        # ---------------- task generators ----------------
        def prep_tasks(j):
            """QK projections + rope + V projection for chunk j, as small tasks."""
            if j >= N_SCHUNK:
                return []
            ssl = slice(j * SC, (j + 1) * SC)
            tasks = []
            state = {}

            def mk_proj(wname, dest, ct):
                def t_first():
                    xt = xt_tiles[j]
                    csl = slice(ct * P, (ct + 1) * P)
                    ps_q = psp.tile([P, SC], f32, tag="ps512", name="psq", bufs=2)
                    state[(wname, ct)] = ps_q
                    for t in range(4):
                        mm(ps_q[:], w_sb[wname][:, t, csl], xt[:, t, :],
                           start=(t == 0), stop=False)

                def t_second():
                    xt = xt_tiles[j]
                    csl = slice(ct * P, (ct + 1) * P)
                    ps_q = state[(wname, ct)]
                    for t in range(4, N_HT):
                        mm(ps_q[:], w_sb[wname][:, t, csl], xt[:, t, :],
                           start=False, stop=(t == N_HT - 1))

                def t_rope():
                    ps_q = state[(wname, ct)]
                    qt = work.tile([P, SC], bf16, tag="qt", name="qt", bufs=3)
                    nc.scalar.copy(qt[:], ps_q[:])
                    ps_rot = psp.tile([P, SC], f32, tag="ps512", name="psrot", bufs=2)
                    mm(ps_rot[:], r2t_sb[:], qt[:], start=True, stop=True)
                    dsl = dest[ct][:, ssl]
                    rt = work.tile([P, SC], bf16, tag="rt", name="rt", bufs=2)
                    eng = getattr(nc, ROPE_COS_ENGINE)
                    nc.vector.tensor_tensor(rt[:], ps_rot[:], sin_sb[:, ssl], mult)
                    eng.tensor_tensor(dsl, qt[:], cos_sb[:, ssl], mult)
                    eng.tensor_tensor(dsl, dsl, rt[:], add)

                return [t_first, t_second, t_rope]

            for (wname, dest) in (("q", qr_sb), ("k", kr_sb)):
                for ct in range(2):
                    tasks.extend(mk_proj(wname, dest, ct))

            def mk_v(st):
                def t_vfirst():
                    xt = xt_tiles[j]
                    ps_v = psp.tile([P, SC], f32, tag="ps512", name="psv", bufs=2)
                    state[("v", st)] = ps_v
                    lsl = slice((st - 4 * j) * P, (st - 4 * j) * P + P)
                    for t in range(4):
                        mm(ps_v[:, 0:C], xt[:, t, lsl], w_sb["v"][:, t, :],
                           start=(t == 0), stop=False)

                def t_vsecond():
                    xt = xt_tiles[j]
                    ps_v = state[("v", st)]
                    lsl = slice((st - 4 * j) * P, (st - 4 * j) * P + P)
                    for t in range(4, N_HT):
                        mm(ps_v[:, 0:C], xt[:, t, lsl], w_sb["v"][:, t, :],
                           start=False, stop=(t == N_HT - 1))
                    nc.vector.tensor_copy(
                        vx_sb[:, st, :, 0:HD],
                        ps_v[:, 0:C].rearrange("p (h d) -> p h d", d=HD))

                return [t_vfirst, t_vsecond]

            for st in range(4 * j, 4 * j + 4):
                tasks.extend(mk_v(st))
            return tasks

        def attn_tasks(j):
            """Attention kt-pair tasks + normalization tasks for chunk j."""
            ssl = slice(j * SC, (j + 1) * SC)
            q0 = j * SC
            n_kt = (q0 + SC) // P
            npair = n_kt // 2
            tasks = []

            for ct in range(2):
                state = {}

                def mk_alloc(ct=ct, state=state):
                    def t_alloc():
                        state["av"] = [
                            psp.tile([HD + 1, SC], f32, tag="av",
                                     name=f"av{hh}", bufs=2)
                            for hh in range(2)]
                    return t_alloc

                def mk_pair(pi, hh, ct=ct, state=state):
                    def t_pair():
                        hsl = slice(hh * HD, (hh + 1) * HD)
                        h = 2 * ct + hh
                        kts = (2 * pi, 2 * pi + 1)
                        ps2 = psp.tile([P, 2, SC], f32, tag="pss",
                                       name="pss", bufs=2)
                        c0s = []
                        for i, kt in enumerate(kts):
                            k0 = kt * P
                            d = (k0 - q0) // P
                            c0 = max(d, 0) * P
                            c0s.append(c0)
                            mm(ps2[:, i, c0:], kr_sb[ct][hsl, k0:k0 + P],
                               qr_sb[ct][hsl, q0 + c0:q0 + SC],
                               start=True, stop=True)
                        pe = work.tile([P, 2, SC], bf16, tag="pe", name="pe",
                                       bufs=3)
                        nc.scalar.activation(pe[:], ps2[:], Exp, scale=0.125)
                        meng = getattr(nc, MASK_ENGINE)
                        for i, kt in enumerate(kts):
                            d = kt - 4 * j
                            if d >= 0:
                                c0 = c0s[i]
                                meng.tensor_tensor(
                                    pe[:, i, c0:c0 + P], pe[:, i, c0:c0 + P],
                                    tri_sb[:], mult)
                        for i, kt in enumerate(kts):
                            c0 = c0s[i]
                            mm(state["av"][hh][:, c0:], vx_sb[:, kt, h, :],
                               pe[:, i, c0:],
                               start=(kt == 0), stop=(kt == n_kt - 1))
                    return t_pair

                def mk_norm(hh, ct=ct, state=state):
                    def t_norm():
                        hsl = slice(hh * HD, (hh + 1) * HD)
                        ps_av = state["av"][hh]
                        rec = work.tile([1, SC], f32r, tag="rec", name="rec",
                                        bufs=2)
                        nc.vector.reciprocal(rec[:], ps_av[HD:HD + 1, :])
                        rb = work.tile([HD, SC], f32r, tag="rb", name="rb",
                                       bufs=2)
                        if USE_PBROADCAST:
                            nc.gpsimd.partition_broadcast(rb[:], rec[:])
                        else:
                            ps_bc = psp.tile([P, SC], f32, tag="ps512",
                                             name="psbc", bufs=2)
                            mm(ps_bc[0:HD, :], ones_sb[:, 0:HD], rec[:],
                               start=True, stop=True)
                            nc.scalar.copy(rb[:], ps_bc[0:HD, :])
                        nc.vector.tensor_tensor(
                            at_sb[ct][hsl, ssl], ps_av[0:HD, :], rb[:], mult)
                    return t_norm

                grp = [mk_alloc()]
                for pi in range(npair):
                    grp.append(mk_pair(pi, 0))
                    grp.append(mk_pair(pi, 1))
                grp.append(mk_norm(0))
                grp.append(mk_norm(1))
                tasks.extend(grp)
            return tasks

        def out_tasks(j):
            tasks = []

            def mk_out(st):
                def t_out():
                    osb = work.tile([P, H], f32, tag="osb", name="osb", bufs=2)
                    osl = slice(st * P, (st + 1) * P)
                    for oc in range(2):
                        ps_o = psp.tile([P, SC], f32, tag="ps512", name="pso",
                                        bufs=2)
                        for ct in range(2):
                            mm(ps_o[:], at_sb[ct][:, osl],
                               wo_sb[ct][:, oc * SC:(oc + 1) * SC],
                               start=(ct == 0), stop=(ct == 1))
                        nc.vector.tensor_copy(osb[:, oc * SC:(oc + 1) * SC],
                                              ps_o[:])
                    nc.sync.dma_start(out=out_d[osl, :], in_=osb[:])
                return t_out

            for st in range(4 * j, 4 * j + 4):
                tasks.append(mk_out(st))
            return tasks

        def weave(stallers, fillers):
            """Emit stallers in order, interleaving fillers evenly between them."""
            nf, ns = len(fillers), len(stallers)
            fi = 0
            for si, t in enumerate(stallers):
                t()
                while fi < nf and (fi + 1) / nf <= (si + 1) / ns:
                    fillers[fi]()
                    fi += 1
            while fi < nf:
                fillers[fi]()
                fi += 1

        def interleave(a, b):
            out = []
            n = max(len(a), len(b))
            for i in range(n):
                if i < len(a):
                    out.append(a[i])
                if i < len(b):
                    out.append(b[i])
            return out

        def body():
            # chunk 0 projections (nothing to overlap with yet)
            for t in prep_tasks(0):
                t()
            for j in range(N_SCHUNK):
                fillers = []
                if j + 1 < N_SCHUNK:
                    if j + 2 < N_SCHUNK:
                        fillers.append(lambda jj=j + 2: load_xt(jj))
                    prev_out = out_tasks(j - 1) if j >= 1 else []
                    fillers.extend(interleave(prep_tasks(j + 1), prev_out))
                else:
                    fillers.extend(out_tasks(j - 1))
                weave(attn_tasks(j), fillers)
            for t in out_tasks(N_SCHUNK - 1):
                t()

        if loop_n is None:
            emit_startup_dmas()
            body()
        else:
            with tc.For_i(0, loop_n, 1):
                emit_startup_dmas()
                body()

        stack.close()

    nc.compile()
    return nc


def _get_program():
    if "nc" not in _PROGRAM_CACHE:
        _PROGRAM_CACHE["nc"] = _build_program()
    return _PROGRAM_CACHE["nc"]


def _host_consts(position_ids_row):
    import ml_dtypes
    inv_freq = 1.0 / (ROPE_THETA ** (np.arange(0, HD, 2, dtype=np.float32) / HD))
    t = position_ids_row.astype(np.float32)
    freqs = t[None, :] * inv_freq[(np.arange(P) % (HD // 2))][:, None]  # [128, S]
    cosT = np.cos(freqs).astype(ml_dtypes.bfloat16)
    sinT = np.sin(freqs).astype(ml_dtypes.bfloat16)
    return np.ascontiguousarray(cosT), np.ascontiguousarray(sinT)


def _make_r2t():
    import ml_dtypes
    R = np.zeros((HD, HD), dtype=np.float32)
    for j in range(HD // 2):
        R[j, j + HD // 2] = -1.0
        R[j + HD // 2, j] = 1.0
    R2 = np.zeros((P, P), dtype=np.float32)
    R2[:HD, :HD] = R
    R2[HD:, HD:] = R
    return np.ascontiguousarray(R2.T.astype(ml_dtypes.bfloat16))


def _make_tri():
    # tri[p, jj] = 1 if jj >= p  (the causal triangle of a diagonal block)
    import ml_dtypes
    jj = np.arange(P)[None, :]
    p = np.arange(P)[:, None]
    return np.ascontiguousarray((jj >= p).astype(ml_dtypes.bfloat16))


def build_in_maps(inputs):
    import ml_dtypes
    hs = np.asarray(inputs["hidden_states"], dtype=np.float32)
    wq = np.asarray(inputs["wq"], dtype=np.float32)
    wk = np.asarray(inputs["wk"], dtype=np.float32)
    wv = np.asarray(inputs["wv"], dtype=np.float32)
    wo = np.asarray(inputs["wo"], dtype=np.float32)
    pos = np.asarray(inputs["position_ids"])

    wqT = np.ascontiguousarray(wq.T)
    wkT = np.ascontiguousarray(wk.T)
    wvT = np.ascontiguousarray(wv.T)
    woT = np.ascontiguousarray(wo.T)
    r2t = _make_r2t()
    tri = _make_tri()
    onesv = np.ones((1, P), dtype=np.float32)
    vones = np.ones((P, N_ST * HEADS_PER_CORE), dtype=ml_dtypes.bfloat16)

    hsT = [np.ascontiguousarray(hs[b].T) for b in range(B)]
    tables = [_host_consts(pos[b]) for b in range(B)]

    in_maps = []
    for core in range(8):
        b, g = divmod(core, GROUPS)
        csl = slice(g * C, (g + 1) * C)
        cosT, sinT = tables[b]
        in_maps.append(dict(
            hsT=hsT[b],
            wqT=np.ascontiguousarray(wqT[:, csl]),
            wkT=np.ascontiguousarray(wkT[:, csl]),
            wvT=np.ascontiguousarray(wvT[:, csl]),
            woT=np.ascontiguousarray(woT[csl, :]),
            cosT=cosT, sinT=sinT, tri=tri, r2t=r2t, onesv=onesv,
            vones=vones,
        ))
    return in_maps


def kernel(**inputs):
    from concourse.bass_utils import run_bass_kernel_spmd

    nc = _get_program()
    in_maps = build_in_maps(inputs)

    res = run_bass_kernel_spmd(nc, in_maps, core_ids=list(range(8)), trace=TRACE)
    global LAST_EXEC_NS
    LAST_EXEC_NS = res.exec_time_ns
    out = np.zeros((B, S, H), dtype=np.float32)
    for core in range(8):
        b = core // GROUPS
        out[b] += res.results[core]["out"]
    return out


# revision 25
# speedup vs baseline: 7.1760x; 7.1760x over previous
"""Trainium2 Bass kernel for causal self-attention with RoPE.

Problem: B=2, S=2048, H=1024, NH=16 heads (HD=64), fp32, causal mask,
rotary embeddings, out = softmax(rope(XWq)(rope(XWk))^T/8 + mask) (XWv) Wo^T.

Sharding (8 cores): tensor-parallel over heads x data-parallel over batch.
core i -> (batch b = i//4, head-group g = i%4 of 4 heads = 256 channels).
Each core computes its group's Q/K/V projections, RoPE, causal attention and
a partial output projection (attnout_g @ Wo_g^T); the host sums the 4 group
partials per batch.

Device-side layout (v2):
 - hidden states fed TRANSPOSED (XT [H, S]); projections contract over the
   partition dim. X/W/Wo/attention-output stay fp32r (full PE rate at the
   512/256-wide moving operands used here).
 - Q^T/K^T [c, s] in bf16 after RoPE; V tiles [k, head, HD+1] bf16 with a
   ones column appended so attn@V also yields the softmax denominator.
 - rotate_half is a constant signed-permutation matmul (bf16); RoPE is
   elementwise in [c, s] split across DVE (psum operand) and Pool (sbuf).
 - scores are computed TRANSPOSED (P^T [ks, qs]) per head into a kt-PAIR
   psum tile [128, 2, 512]; ONE exp (scale=0.125, no max-subtraction:
   scores are bounded ~ +-5 for this problem) covers both kt blocks,
   halving Activation-engine instruction count. exp output is bf16.
 - causal masking multiplies only the 128-wide diagonal triangle of each
   diagonal kt block (everything right of it is already causal); the
   triangle mask is one [128,128] table shared by all blocks.
 - softmax normalization: reciprocal of the denominator row (DVE), then a
   gpsimd partition_broadcast replicates it across 64 partitions, then one
   DVE tensor-tensor multiply writes the normalized attention output.
 - attention output [hd, qs] is exactly the lhsT the output projection
   wants; out = at^T wo accumulated over the 2 half-channel groups.
"""
import sys
import numpy as np

sys.path.insert(0, '/opt/trn_rl_repo')

B, S, H, NH, HD = 2, 2048, 1024, 16, 64
GROUPS = 4            # head-groups (cores per batch)
HEADS_PER_CORE = 4
C = HEADS_PER_CORE * HD   # 256 channels per core
P = 128               # partitions
SC = 512              # s-chunk (matmul free dim)
N_SCHUNK = S // SC    # 4
N_HT = H // P         # 8 h-tiles
N_ST = S // P         # 16 s-tiles
ROPE_THETA = 10000.0

_PROGRAM_CACHE = {}
TRACE = False          # set True (e.g. from test.py) to profile and fill LAST_EXEC_NS
LAST_EXEC_NS = None

USE_PBROADCAST = True  # gpsimd partition_broadcast for softmax normalization
MASK_ENGINE = "vector"    # engine for the diagonal triangle mask multiply
ROPE_COS_ENGINE = "gpsimd"  # engine for rope's qt*cos and +=rt ops


def _build_program(loop_n=None):
    import concourse.bass as bass
    import concourse.mybir as mybir
    import concourse.tile as tile
    from concourse import bacc

    f32 = mybir.dt.float32
    f32r = mybir.dt.float32r
    bf16 = mybir.dt.bfloat16
    Exp = mybir.ActivationFunctionType.Exp
    mult = mybir.AluOpType.mult
    add = mybir.AluOpType.add

    nc = bacc.Bacc("TRN2", target_bir_lowering=False, debug=False, num_devices=8)

    # ---- DRAM parameters (per-core inputs) ----
    hsT = nc.declare_dram_parameter("hsT", [H, S], f32, isOutput=False)
    wqT = nc.declare_dram_parameter("wqT", [H, C], f32, isOutput=False)
    wkT = nc.declare_dram_parameter("wkT", [H, C], f32, isOutput=False)
    wvT = nc.declare_dram_parameter("wvT", [H, C], f32, isOutput=False)
    woT = nc.declare_dram_parameter("woT", [C, H], f32, isOutput=False)
    cosT = nc.declare_dram_parameter("cosT", [P, S], bf16, isOutput=False)
    sinT = nc.declare_dram_parameter("sinT", [P, S], bf16, isOutput=False)
    tri = nc.declare_dram_parameter("tri", [P, P], bf16, isOutput=False)
    r2t = nc.declare_dram_parameter("r2t", [P, P], bf16, isOutput=False)
    onesv = nc.declare_dram_parameter("onesv", [1, P], f32, isOutput=False)
    vones = nc.declare_dram_parameter("vones", [P, N_ST * HEADS_PER_CORE], bf16,
                                      isOutput=False)
    out_d = nc.declare_dram_parameter("out", [S, H], f32, isOutput=True)

    mm = nc.tensor.matmul

    with tile.TileContext(nc) as tc, \
         nc.allow_low_precision(reason="bf16 attention internals; "
                                "accumulation stays fp32 in PSUM"):
        import contextlib
        stack = contextlib.ExitStack()
        persist = stack.enter_context(tc.tile_pool(name="persist", bufs=1))
        work = stack.enter_context(tc.tile_pool(name="work", bufs=1))
        psp = stack.enter_context(tc.tile_pool(name="psp", bufs=1, space="PSUM"))

        # ---------------- persistent tiles ----------------
        cos_sb = persist.tile([P, S], bf16, tag="cos")
        sin_sb = persist.tile([P, S], bf16, tag="sin")
        tri_sb = persist.tile([P, P], bf16, tag="tri")
        r2t_sb = persist.tile([P, P], bf16, tag="r2t")
        ones_sb = persist.tile([1, P], f32r, tag="ones")
        wo_sb = [persist.tile([P, H], f32r, tag=f"wo{i}", name=f"wo{i}") for i in range(2)]
        qr_sb = [persist.tile([P, S], bf16, tag=f"qr{i}", name=f"qr{i}") for i in range(2)]
        kr_sb = [persist.tile([P, S], bf16, tag=f"kr{i}", name=f"kr{i}") for i in range(2)]
        vx_sb = persist.tile([P, N_ST, HEADS_PER_CORE, HD + 1], bf16, tag="vx")
        at_sb = [persist.tile([P, S], f32r, tag=f"at{i}", name=f"at{i}") for i in range(2)]
        w_sb = {n: work.tile([P, N_HT, C], f32r, tag=f"w{n}", name=f"w{n}")
                for n in ("q", "k", "v")}

        xt_tiles = {}

        def load_xt(j, halves=(0, 1)):
            # one DMA per half chunk: [128, 4 h-tiles, 512] (rotating, bufs=2)
            if j in xt_tiles:
                x = xt_tiles[j]
            else:
                x = work.tile([P, N_HT, SC], f32r, tag="xt", name=f"xt_{j}",
                              bufs=2)
                xt_tiles[j] = x
            src = hsT.ap().rearrange("(t p) s -> p t s", p=P)
            for hf in halves:
                tsl = slice(hf * 4, hf * 4 + 4)
                nc.sync.dma_start(
                    out=x[:, tsl, :],
                    in_=src[:, tsl, j * SC:(j + 1) * SC].bitcast(f32r))

        # ---- startup DMAs: q-weights + first x chunk first, then the rest ----
        wdram = {"q": wqT, "k": wkT, "v": wvT}
        def emit_startup_dmas():
            def one_w(n, halves=(0, 1)):
                src = wdram[n].ap().rearrange("(t p) c -> p t c", p=P)
                for hf in halves:
                    tsl = slice(hf * 4, hf * 4 + 4)
                    nc.sync.dma_start(out=w_sb[n][:, tsl, :],
                                      in_=src[:, tsl, :].bitcast(f32r))
            # q projections need wq + the x chunk; rope needs the tables;
            # then k; xt(1) before wv so prep(1) can fill the startup hole.
            one_w("q", halves=(0,))
            load_xt(0, halves=(0,))
            one_w("q", halves=(1,))
            load_xt(0, halves=(1,))
            nc.sync.dma_start(out=r2t_sb[:], in_=r2t[:])
            nc.sync.dma_start(out=cos_sb[:], in_=cosT[:])
            nc.sync.dma_start(out=sin_sb[:], in_=sinT[:])
            one_w("k")
            nc.sync.dma_start(out=tri_sb[:], in_=tri[:])
            load_xt(1)
            nc.sync.dma_start(
                out=vx_sb[:, :, :, HD],
                in_=vones.ap().rearrange("p (st h) -> p st h", h=HEADS_PER_CORE))
            one_w("v")
            nc.sync.dma_start(out=ones_sb[:], in_=onesv.ap().bitcast(f32r))
            for i in range(2):
                nc.sync.dma_start(out=wo_sb[i][:],
                                  in_=woT[i * P:(i + 1) * P, :].bitcast(f32r))

        # ---------------- task generators ----------------
        def prep_tasks(j):
            """QK projections + rope + V projection for chunk j, as small tasks."""
            if j >= N_SCHUNK:
                return []
            ssl = slice(j * SC, (j + 1) * SC)
            tasks = []
            state = {}

            def mk_proj(wname, dest, ct):
                def t_first():
                    xt = xt_tiles[j]
                    csl = slice(ct * P, (ct + 1) * P)
                    ps_q = psp.tile([P, SC], f32, tag="ps512", name="psq", bufs=2)
                    state[(wname, ct)] = ps_q
                    for t in range(4):
                        mm(ps_q[:], w_sb[wname][:, t, csl], xt[:, t, :],
                           start=(t == 0), stop=False)

                def t_second():
                    xt = xt_tiles[j]
                    csl = slice(ct * P, (ct + 1) * P)
                    ps_q = state[(wname, ct)]
                    for t in range(4, N_HT):
                        mm(ps_q[:], w_sb[wname][:, t, csl], xt[:, t, :],
                           start=False, stop=(t == N_HT - 1))

                def t_rope():
                    ps_q = state[(wname, ct)]
                    qt = work.tile([P, SC], bf16, tag="qt", name="qt", bufs=3)
                    nc.scalar.copy(qt[:], ps_q[:])
                    ps_rot = psp.tile([P, SC], f32, tag="ps512", name="psrot", bufs=2)
                    mm(ps_rot[:], r2t_sb[:], qt[:], start=True, stop=True)
                    dsl = dest[ct][:, ssl]
                    rt = work.tile([P, SC], bf16, tag="rt", name="rt", bufs=2)
                    eng = getattr(nc, ROPE_COS_ENGINE)
                    nc.vector.tensor_tensor(rt[:], ps_rot[:], sin_sb[:, ssl], mult)
                    eng.tensor_tensor(dsl, qt[:], cos_sb[:, ssl], mult)
                    eng.tensor_tensor(dsl, dsl, rt[:], add)

                return [t_first, t_second, t_rope]

            for (wname, dest) in (("q", qr_sb), ("k", kr_sb)):
                for ct in range(2):
                    tasks.extend(mk_proj(wname, dest, ct))

            def mk_v(st):
                def t_vfirst():
                    xt = xt_tiles[j]
                    ps_v = psp.tile([P, SC], f32, tag="ps512", name="psv", bufs=2)
                    state[("v", st)] = ps_v
                    lsl = slice((st - 4 * j) * P, (st - 4 * j) * P + P)
                    for t in range(4):
                        mm(ps_v[:, 0:C], xt[:, t, lsl], w_sb["v"][:, t, :],
                           start=(t == 0), stop=False)

                def t_vsecond():
                    xt = xt_tiles[j]
                    ps_v = state[("v", st)]
                    lsl = slice((st - 4 * j) * P, (st - 4 * j) * P + P)
                    for t in range(4, N_HT):
                        mm(ps_v[:, 0:C], xt[:, t, lsl], w_sb["v"][:, t, :],
                           start=False, stop=(t == N_HT - 1))
                    nc.vector.tensor_copy(
                        vx_sb[:, st, :, 0:HD],
                        ps_v[:, 0:C].rearrange("p (h d) -> p h d", d=HD))

                return [t_vfirst, t_vsecond]

            for st in range(4 * j, 4 * j + 4):
                tasks.extend(mk_v(st))
            return tasks

        def attn_tasks(j):
            """Attention kt-pair tasks + normalization tasks for chunk j."""
            ssl = slice(j * SC, (j + 1) * SC)
            q0 = j * SC
            n_kt = (q0 + SC) // P
            npair = n_kt // 2
            tasks = []

            for ct in range(2):
                state = {}

                def mk_alloc(ct=ct, state=state):
                    def t_alloc():
                        state["av"] = [
                            psp.tile([HD + 1, SC], f32, tag="av",
                                     name=f"av{hh}", bufs=2)
                            for hh in range(2)]
                    return t_alloc

                def mk_pair(pi, hh, ct=ct, state=state):
                    def t_pair():
                        hsl = slice(hh * HD, (hh + 1) * HD)
                        h = 2 * ct + hh
                        kts = (2 * pi, 2 * pi + 1)
                        ps2 = psp.tile([P, 2, SC], f32, tag="pss",
                                       name="pss", bufs=2)
                        c0s = []
                        for i, kt in enumerate(kts):
                            k0 = kt * P
                            d = (k0 - q0) // P
                            c0 = max(d, 0) * P
                            c0s.append(c0)
                            mm(ps2[:, i, c0:], kr_sb[ct][hsl, k0:k0 + P],
                               qr_sb[ct][hsl, q0 + c0:q0 + SC],
                               start=True, stop=True)
                        pe = work.tile([P, 2, SC], bf16, tag="pe", name="pe",
                                       bufs=3)
                        nc.scalar.activation(pe[:], ps2[:], Exp, scale=0.125)
                        meng = getattr(nc, MASK_ENGINE)
                        for i, kt in enumerate(kts):
                            d = kt - 4 * j
                            if d >= 0:
                                c0 = c0s[i]
                                meng.tensor_tensor(
                                    pe[:, i, c0:c0 + P], pe[:, i, c0:c0 + P],
                                    tri_sb[:], mult)
                        for i, kt in enumerate(kts):
                            c0 = c0s[i]
                            mm(state["av"][hh][:, c0:], vx_sb[:, kt, h, :],
                               pe[:, i, c0:],
                               start=(kt == 0), stop=(kt == n_kt - 1))
                    return t_pair

                def mk_norm(hh, ct=ct, state=state):
                    def t_norm():
                        hsl = slice(hh * HD, (hh + 1) * HD)
                        ps_av = state["av"][hh]
                        rec = work.tile([1, SC], f32r, tag="rec", name="rec",
                                        bufs=2)
                        nc.vector.reciprocal(rec[:], ps_av[HD:HD + 1, :])
                        rb = work.tile([HD, SC], f32r, tag="rb", name="rb",
                                       bufs=2)
                        if USE_PBROADCAST:
                            nc.gpsimd.partition_broadcast(rb[:], rec[:])
                        else:
                            ps_bc = psp.tile([P, SC], f32, tag="ps512",
                                             name="psbc", bufs=2)
                            mm(ps_bc[0:HD, :], ones_sb[:, 0:HD], rec[:],
                               start=True, stop=True)
                            nc.scalar.copy(rb[:], ps_bc[0:HD, :])
                        nc.vector.tensor_tensor(
                            at_sb[ct][hsl, ssl], ps_av[0:HD, :], rb[:], mult)
                    return t_norm

                grp = [mk_alloc()]
                for pi in range(npair):
                    grp.append(mk_pair(pi, 0))
                    grp.append(mk_pair(pi, 1))
                grp.append(mk_norm(0))
                grp.append(mk_norm(1))
                tasks.extend(grp)
            return tasks

        def out_tasks(j):
            tasks = []

            def mk_out(st):
                def t_out():
                    osb = work.tile([P, H], f32, tag="osb", name="osb", bufs=2)
                    osl = slice(st * P, (st + 1) * P)
                    for oc in range(2):
                        ps_o = psp.tile([P, SC], f32, tag="ps512", name="pso",
                                        bufs=2)
                        for ct in range(2):
                            mm(ps_o[:], at_sb[ct][:, osl],
                               wo_sb[ct][:, oc * SC:(oc + 1) * SC],
                               start=(ct == 0), stop=(ct == 1))
                        nc.vector.tensor_copy(osb[:, oc * SC:(oc + 1) * SC],
                                              ps_o[:])
                    nc.sync.dma_start(out=out_d[osl, :], in_=osb[:])
                return t_out

            for st in range(4 * j, 4 * j + 4):
                tasks.append(mk_out(st))
            return tasks

        def weave(stallers, fillers):
            """Emit stallers in order, interleaving fillers evenly between them."""
            nf, ns = len(fillers), len(stallers)
            fi = 0
            for si, t in enumerate(stallers):
                t()
                while fi < nf and (fi + 1) / nf <= (si + 1) / ns:
                    fillers[fi]()
                    fi += 1
            while fi < nf:
                fillers[fi]()
                fi += 1

        def interleave(a, b):
            out = []
            n = max(len(a), len(b))
            for i in range(n):
                if i < len(a):
                    out.append(a[i])
                if i < len(b):
                    out.append(b[i])
            return out

        def body():
            # chunk 0 projections (nothing to overlap with yet)
            for t in prep_tasks(0):
                t()
            for j in range(N_SCHUNK):
                fillers = []
                if j + 2 < N_SCHUNK:
                    fillers.append(lambda jj=j + 2: load_xt(jj))
                nxt = prep_tasks(j + 1) if j + 1 < N_SCHUNK else []
                # out tasks are deferred one extra chunk so the filler-poor
                # last chunk (longest kt chains, no prep left) gets two
                # chunks' worth of output-projection work to hide its
                # norm barriers
                if j == 2:
                    outs = out_tasks(0)
                elif j == 3:
                    outs = out_tasks(1) + out_tasks(2)
                else:
                    outs = []
                fillers.extend(interleave(nxt, outs))
                weave(attn_tasks(j), fillers)
            for t in out_tasks(N_SCHUNK - 1):
                t()

        if loop_n is None:
            emit_startup_dmas()
            body()
        else:
            with tc.For_i(0, loop_n, 1):
                emit_startup_dmas()
                body()

        stack.close()

    nc.compile()
    return nc


def _get_program():
    if "nc" not in _PROGRAM_CACHE:
        _PROGRAM_CACHE["nc"] = _build_program()
    return _PROGRAM_CACHE["nc"]


def _host_consts(position_ids_row):
    import ml_dtypes
    inv_freq = 1.0 / (ROPE_THETA ** (np.arange(0, HD, 2, dtype=np.float32) / HD))
    t = position_ids_row.astype(np.float32)
    freqs = t[None, :] * inv_freq[(np.arange(P) % (HD // 2))][:, None]  # [128, S]
    cosT = np.cos(freqs).astype(ml_dtypes.bfloat16)
    sinT = np.sin(freqs).astype(ml_dtypes.bfloat16)
    return np.ascontiguousarray(cosT), np.ascontiguousarray(sinT)


def _make_r2t():
    import ml_dtypes
    R = np.zeros((HD, HD), dtype=np.float32)
    for j in range(HD // 2):
        R[j, j + HD // 2] = -1.0
        R[j + HD // 2, j] = 1.0
    R2 = np.zeros((P, P), dtype=np.float32)
    R2[:HD, :HD] = R
    R2[HD:, HD:] = R
    return np.ascontiguousarray(R2.T.astype(ml_dtypes.bfloat16))


def _make_tri():
    # tri[p, jj] = 1 if jj >= p  (the causal triangle of a diagonal block)
    import ml_dtypes
    jj = np.arange(P)[None, :]
    p = np.arange(P)[:, None]
    return np.ascontiguousarray((jj >= p).astype(ml_dtypes.bfloat16))


def build_in_maps(inputs):
    import ml_dtypes
    hs = np.asarray(inputs["hidden_states"], dtype=np.float32)
    wq = np.asarray(inputs["wq"], dtype=np.float32)
    wk = np.asarray(inputs["wk"], dtype=np.float32)
    wv = np.asarray(inputs["wv"], dtype=np.float32)
    wo = np.asarray(inputs["wo"], dtype=np.float32)
    pos = np.asarray(inputs["position_ids"])

    wqT = np.ascontiguousarray(wq.T)
    wkT = np.ascontiguousarray(wk.T)
    wvT = np.ascontiguousarray(wv.T)
    woT = np.ascontiguousarray(wo.T)
    r2t = _make_r2t()
    tri = _make_tri()
    onesv = np.ones((1, P), dtype=np.float32)
    vones = np.ones((P, N_ST * HEADS_PER_CORE), dtype=ml_dtypes.bfloat16)

    hsT = [np.ascontiguousarray(hs[b].T) for b in range(B)]
    tables = [_host_consts(pos[b]) for b in range(B)]

    in_maps = []
    for core in range(8):
        b, g = divmod(core, GROUPS)
        csl = slice(g * C, (g + 1) * C)
        cosT, sinT = tables[b]
        in_maps.append(dict(
            hsT=hsT[b],
            wqT=np.ascontiguousarray(wqT[:, csl]),
            wkT=np.ascontiguousarray(wkT[:, csl]),
            wvT=np.ascontiguousarray(wvT[:, csl]),
            woT=np.ascontiguousarray(woT[csl, :]),
            cosT=cosT, sinT=sinT, tri=tri, r2t=r2t, onesv=onesv,
            vones=vones,
        ))
    return in_maps


def kernel(**inputs):
    from concourse.bass_utils import run_bass_kernel_spmd

    nc = _get_program()
    in_maps = build_in_maps(inputs)

    res = run_bass_kernel_spmd(nc, in_maps, core_ids=list(range(8)), trace=TRACE)
    global LAST_EXEC_NS
    LAST_EXEC_NS = res.exec_time_ns
    out = np.zeros((B, S, H), dtype=np.float32)
    for core in range(8):
        b = core // GROUPS
        out[b] += res.results[core]["out"]
    return out
